# revision 14
# baseline (speedup 1.0000x reference)
"""CBOW negative-sampling loss on 8 Trainium2 NeuronCores.

Strategy: data-parallel over the batch. Each core processes B/8 = 2048
examples against fully-replicated embedding tables (tables live in each
core's HBM; all "lookups" are indirect-DMA row gathers, so the kernel is
HBM-bandwidth bound: ~20 MB of gathered rows per core).

Per core, examples are laid out one-per-partition in tiles of 128
(example t*128+p -> partition p, tile t; 16 tiles). Tiles are processed
in chunks of CT. Index regions are packed on the host so that each
gather's destination layout makes every DVE op a flat 2-dim AP
(extended >3-dim TT encodings can't carry the required sync waits):
  - ctx rows land position-major [P, CTX, CT, D]: the 8-way context sum
    is 3 in-place folds of contiguous halves.
  - neg rows land k-major [P, K, CT, D]: v multiplies each k-slab as a
    flat [P, CT*D] op.
Loss = -ln sig(s_pos/8) - sum_k ln sig(-s_neg_k/8) via ACT sigmoid+ln
(the 1/8 folds the ctx mean into the activation scale), then a negated
X-axis reduce over the 11 slots.
"""
import numpy as np

import concourse.bacc as bacc
import concourse.bass as bass
import concourse.mybir as mybir
from concourse.bass import IndirectOffsetOnAxis
from concourse.bass_utils import run_bass_kernel_spmd
from concourse.tile import TileContext

P = 128
VOCAB, D = 100000, 128
B, CTX, K = 16384, 8, 10
NCORES = 8
B_SHARD = B // NCORES          # 2048
NTILES = B_SHARD // P          # 16
CT = 1                         # tiles per chunk
F32 = mybir.dt.float32
I32 = mybir.dt.int32



_QN = [0]


def _q(inst):
    """Round-robin SWDGE queue assignment for indirect DMAs."""
    qi = _QN[0] % 4
    _QN[0] += 1
    if qi:
        inst.ins.queue = f"qPoolDynamic{qi}"
    return inst


def build(vocab=VOCAB, ntiles=NTILES, ct=CT, loop_n=None) -> bass.Bass:
    """loop_n: if set, wrap the whole body in a device-side repeat loop
    (benchmarking only — output is idempotent)."""
    from contextlib import nullcontext

    nchunk = ntiles // ct
    cw = ct * (CTX + 1 + K)        # idx cols per chunk
    off_tgt = ct * CTX             # within-chunk offsets
    off_neg = ct * (CTX + 1)
    nidx = ntiles * (CTX + 1 + K)

    nc = bacc.Bacc("TRN2", target_bir_lowering=False, debug=False,
                   num_devices=NCORES, num_swdge_queues=4)
    in_embed = nc.dram_tensor("in_embed", [vocab, D], F32, kind="ExternalInput")
    out_embed = nc.dram_tensor("out_embed", [vocab, D], F32, kind="ExternalInput")
    idx = nc.dram_tensor("idx", [P, nidx], I32, kind="ExternalInput")
    loss = nc.dram_tensor("loss", [P, ntiles], F32, kind="ExternalOutput")

    with TileContext(nc) as tc:
        with (
            tc.tile_pool(name="idxp", bufs=6) as ipool,
            tc.tile_pool(name="work", bufs=6) as work,
        ):
            loop_cm = tc.For_i(0, loop_n, 1) if loop_n else nullcontext()
            with loop_cm:
                for c in range(nchunk):
                    idx_t = ipool.tile([P, cw], I32, tag="idx")
                    nc.sync.dma_start(out=idx_t[:],
                                      in_=idx[:, c * cw:(c + 1) * cw])
                    ctx_g = work.tile([P, CTX * ct * D], F32, tag="ctx")
                    pos_g = work.tile([P, ct * D], F32, tag="pos")
                    neg_g = work.tile([P, K * ct * D], F32, tag="neg")

                    # gathers: HW indirect DMA honors exactly one index per
                    # partition per op, so issue one [P,1]->[P,D] gather per
                    # destination row-slot. idx regions are packed to match.
                    for j in range(ct * CTX):
                        _q(nc.gpsimd.indirect_dma_start(
                            out=ctx_g[:, j * D:(j + 1) * D], out_offset=None,
                            in_=in_embed[:],
                            in_offset=IndirectOffsetOnAxis(
                                ap=idx_t[:, j:j + 1], axis=0)))
                    for j in range(ct):
                        _q(nc.gpsimd.indirect_dma_start(
                            out=pos_g[:, j * D:(j + 1) * D], out_offset=None,
                            in_=out_embed[:],
                            in_offset=IndirectOffsetOnAxis(
                                ap=idx_t[:, off_tgt + j:off_tgt + j + 1],
                                axis=0)))
                    for j in range(ct * K):
                        _q(nc.gpsimd.indirect_dma_start(
                            out=neg_g[:, j * D:(j + 1) * D], out_offset=None,
                            in_=out_embed[:],
                            in_offset=IndirectOffsetOnAxis(
                                ap=idx_t[:, off_neg + j:off_neg + j + 1],
                                axis=0)))

                    # v_sum: fold contiguous halves (position-major layout)
                    w = ct * D
                    for half in (4, 2, 1):
                        nc.vector.tensor_add(
                            out=ctx_g[:, 0:half * w],
                            in0=ctx_g[:, 0:half * w],
                            in1=ctx_g[:, half * w:2 * half * w])
                    v = ctx_g[:, 0:w]  # [P, ct*D] contiguous

                    # pos scores
                    nc.vector.tensor_mul(out=pos_g[:], in0=pos_g[:], in1=v)
                    s_pos = work.tile([P, ct], F32, tag="spos")
                    nc.vector.reduce_sum(
                        out=s_pos[:],
                        in_=pos_g[:].rearrange("p (t d) -> p t d", d=D),
                        axis=mybir.AxisListType.X)

                    # neg scores: one flat mul per k (k-major layout)
                    for k in range(K):
                        nc.vector.tensor_mul(
                            out=neg_g[:, k * w:(k + 1) * w],
                            in0=neg_g[:, k * w:(k + 1) * w], in1=v)
                    s_neg = work.tile([P, K * ct], F32, tag="sneg")
                    nc.vector.reduce_sum(
                        out=s_neg[:],
                        in_=neg_g[:].rearrange("p (k d) -> p k d", d=D),
                        axis=mybir.AxisListType.X)

                    # sig_all layout [P, (1+K), ct]: pos slab then k slabs
                    sig_all = work.tile([P, (K + 1) * ct], F32, tag="sig")
                    nc.scalar.activation(
                        out=sig_all[:, 0:ct], in_=s_pos[:],
                        func=mybir.ActivationFunctionType.Sigmoid, scale=1.0 / CTX)
                    nc.scalar.activation(
                        out=sig_all[:, ct:(K + 1) * ct], in_=s_neg[:],
                        func=mybir.ActivationFunctionType.Sigmoid, scale=-1.0 / CTX)
                    nc.scalar.activation(
                        out=sig_all[:], in_=sig_all[:],
                        func=mybir.ActivationFunctionType.Ln)

                    # loss[p, t] = -sum_j sig_all[p, j, t]
                    loss_t = work.tile([P, ct], F32, tag="losst")
                    nc.vector.tensor_reduce(
                        out=loss_t[:],
                        in_=sig_all[:].rearrange("p (j t) -> p j t", t=ct)
                            .transpose([0, 2, 1]),
                        op=mybir.AluOpType.add,
                        axis=mybir.AxisListType.X, negate=True)
                    nc.sync.dma_start(
                        out=loss[:, c * ct:(c + 1) * ct], in_=loss_t[:])
    nc.finalize()
    return nc


def _pack_core_idx(context, target, negatives, ntiles=NTILES, ct=CT):
    """[B_shard,*] int arrays -> [P, nidx] i32.

    Example (c*ct + t)*P + p lives at partition p, chunk c, tile-slot t.
    ctx region per chunk is position-major [CTX, ct]; tgt is [ct];
    neg region per chunk is k-major [K, ct].
    """
    nchunk = ntiles // ct
    ctx_idx = (context.reshape(nchunk, ct, P, CTX)
               .transpose(2, 0, 3, 1))             # [P, nchunk, CTX, ct]
    tgt_idx = target.reshape(nchunk, ct, P).transpose(2, 0, 1)  # [P, nchunk, ct]
    neg_idx = (negatives.reshape(nchunk, ct, P, K)
               .transpose(2, 0, 3, 1))             # [P, nchunk, K, ct]
    blocks = np.concatenate(
        [ctx_idx.reshape(P, nchunk, CTX * ct),
         tgt_idx.reshape(P, nchunk, ct),
         neg_idx.reshape(P, nchunk, K * ct)], axis=2)  # [P, nchunk, cw]
    return np.ascontiguousarray(
        blocks.reshape(P, ntiles * (CTX + 1 + K)).astype(np.int32))


def _run(inputs, trace=False):
    in_embed = np.ascontiguousarray(np.asarray(inputs["in_embed"], dtype=np.float32))
    out_embed = np.ascontiguousarray(np.asarray(inputs["out_embed"], dtype=np.float32))
    context = np.asarray(inputs["context"]).astype(np.int32)
    target = np.asarray(inputs["target"]).astype(np.int32)
    negatives = np.asarray(inputs["negatives"]).astype(np.int32)
    assert context.shape == (B, CTX) and target.shape == (B,) and negatives.shape == (B, K)

    nc = build()
    in_maps = []
    for i in range(NCORES):
        sl = slice(i * B_SHARD, (i + 1) * B_SHARD)
        in_maps.append({
            "in_embed": in_embed,
            "out_embed": out_embed,
            "idx": _pack_core_idx(context[sl], target[sl], negatives[sl]),
        })
    res = run_bass_kernel_spmd(nc, in_maps, core_ids=list(range(NCORES)),
                               trace=trace)
    loss = np.concatenate(
        [res.results[i]["loss"].T.reshape(-1) for i in range(NCORES)])
    return loss.astype(np.float32), res


def kernel(**inputs) -> np.ndarray:
    return _run(inputs, trace=False)[0]


# revision 15
# speedup vs baseline: 1.0073x; 1.0073x over previous
"""CBOW negative-sampling loss on 8 Trainium2 NeuronCores.

Strategy: data-parallel over the batch. Each core processes B/8 = 2048
examples against fully-replicated embedding tables (tables live in each
core's HBM; all "lookups" are indirect-DMA row gathers, so the kernel is
HBM-bandwidth bound: ~20 MB of gathered rows per core).

Per core, examples are laid out one-per-partition in tiles of 128
(example t*128+p -> partition p, tile t; 16 tiles). Tiles are processed
in chunks of CT. Index regions are packed on the host so that each
gather's destination layout makes every DVE op a flat 2-dim AP
(extended >3-dim TT encodings can't carry the required sync waits):
  - ctx rows land position-major [P, CTX, CT, D]: the 8-way context sum
    is 3 in-place folds of contiguous halves.
  - neg rows land k-major [P, K, CT, D]: v multiplies each k-slab as a
    flat [P, CT*D] op.
Loss = -ln sig(s_pos/8) - sum_k ln sig(-s_neg_k/8) via ACT sigmoid+ln
(the 1/8 folds the ctx mean into the activation scale), then a negated
X-axis reduce over the 11 slots.
"""
import numpy as np

import concourse.bacc as bacc
import concourse.bass as bass
import concourse.mybir as mybir
from concourse.bass import IndirectOffsetOnAxis
from concourse.bass_utils import run_bass_kernel_spmd
from concourse.tile import TileContext

P = 128
VOCAB, D = 100000, 128
B, CTX, K = 16384, 8, 10
NCORES = 8
B_SHARD = B // NCORES          # 2048
NTILES = B_SHARD // P          # 16
CT = 1                         # tiles per chunk
F32 = mybir.dt.float32
I32 = mybir.dt.int32



_QN = [0]


def _q(inst):
    """Round-robin SWDGE queue assignment for indirect DMAs."""
    qi = _QN[0] % 4
    _QN[0] += 1
    if qi:
        inst.ins.queue = f"qPoolDynamic{qi}"
    return inst


def build(vocab=VOCAB, ntiles=NTILES, ct=CT, loop_n=None) -> bass.Bass:
    """loop_n: if set, wrap the whole body in a device-side repeat loop
    (benchmarking only — output is idempotent)."""
    from contextlib import nullcontext

    nchunk = ntiles // ct
    off_tgt = ntiles * CTX
    off_neg = ntiles * (CTX + 1)
    nidx = ntiles * (CTX + 1 + K)

    nc = bacc.Bacc("TRN2", target_bir_lowering=False, debug=False,
                   num_devices=NCORES, num_swdge_queues=4)
    in_embed = nc.dram_tensor("in_embed", [vocab, D], F32, kind="ExternalInput")
    out_embed = nc.dram_tensor("out_embed", [vocab, D], F32, kind="ExternalInput")
    idx = nc.dram_tensor("idx", [P, nidx], I32, kind="ExternalInput")
    loss = nc.dram_tensor("loss", [P, ntiles], F32, kind="ExternalOutput")

    with TileContext(nc) as tc:
        with (
            tc.tile_pool(name="const", bufs=1) as cpool,
            tc.tile_pool(name="work", bufs=6) as work,
        ):
            idx_t = cpool.tile([P, nidx], I32)
            nc.sync.dma_start(out=idx_t[:], in_=idx[:])

            loop_cm = tc.For_i(0, loop_n, 1) if loop_n else nullcontext()
            with loop_cm:
                for c in range(nchunk):
                    ctx_g = work.tile([P, CTX * ct * D], F32, tag="ctx")
                    pos_g = work.tile([P, ct * D], F32, tag="pos")
                    neg_g = work.tile([P, K * ct * D], F32, tag="neg")

                    # gathers: HW indirect DMA honors exactly one index per
                    # partition per op, so issue one [P,1]->[P,D] gather per
                    # destination row-slot. idx regions are packed to match.
                    for j in range(ct * CTX):
                        _q(nc.gpsimd.indirect_dma_start(
                            out=ctx_g[:, j * D:(j + 1) * D], out_offset=None,
                            in_=in_embed[:],
                            in_offset=IndirectOffsetOnAxis(
                                ap=idx_t[:, c * ct * CTX + j:
                                         c * ct * CTX + j + 1], axis=0)))
                    for j in range(ct):
                        _q(nc.gpsimd.indirect_dma_start(
                            out=pos_g[:, j * D:(j + 1) * D], out_offset=None,
                            in_=out_embed[:],
                            in_offset=IndirectOffsetOnAxis(
                                ap=idx_t[:, off_tgt + c * ct + j:
                                         off_tgt + c * ct + j + 1], axis=0)))
                    for j in range(ct * K):
                        _q(nc.gpsimd.indirect_dma_start(
                            out=neg_g[:, j * D:(j + 1) * D], out_offset=None,
                            in_=out_embed[:],
                            in_offset=IndirectOffsetOnAxis(
                                ap=idx_t[:, off_neg + c * ct * K + j:
                                         off_neg + c * ct * K + j + 1], axis=0)))

                    # v_sum: fold contiguous halves (position-major layout)
                    w = ct * D
                    for half in (4, 2, 1):
                        nc.vector.tensor_add(
                            out=ctx_g[:, 0:half * w],
                            in0=ctx_g[:, 0:half * w],
                            in1=ctx_g[:, half * w:2 * half * w])
                    v = ctx_g[:, 0:w]  # [P, ct*D] contiguous

                    # pos scores
                    nc.vector.tensor_mul(out=pos_g[:], in0=pos_g[:], in1=v)
                    s_pos = work.tile([P, ct], F32, tag="spos")
                    nc.vector.reduce_sum(
                        out=s_pos[:],
                        in_=pos_g[:].rearrange("p (t d) -> p t d", d=D),
                        axis=mybir.AxisListType.X)

                    # neg scores: one flat mul per k (k-major layout)
                    for k in range(K):
                        nc.vector.tensor_mul(
                            out=neg_g[:, k * w:(k + 1) * w],
                            in0=neg_g[:, k * w:(k + 1) * w], in1=v)
                    s_neg = work.tile([P, K * ct], F32, tag="sneg")
                    nc.vector.reduce_sum(
                        out=s_neg[:],
                        in_=neg_g[:].rearrange("p (k d) -> p k d", d=D),
                        axis=mybir.AxisListType.X)

                    # sig_all layout [P, (1+K), ct]: pos slab then k slabs
                    sig_all = work.tile([P, (K + 1) * ct], F32, tag="sig")
                    nc.scalar.activation(
                        out=sig_all[:, 0:ct], in_=s_pos[:],
                        func=mybir.ActivationFunctionType.Sigmoid, scale=1.0 / CTX)
                    nc.scalar.activation(
                        out=sig_all[:, ct:(K + 1) * ct], in_=s_neg[:],
                        func=mybir.ActivationFunctionType.Sigmoid, scale=-1.0 / CTX)
                    nc.scalar.activation(
                        out=sig_all[:], in_=sig_all[:],
                        func=mybir.ActivationFunctionType.Ln)

                    # loss[p, t] = -sum_j sig_all[p, j, t]
                    loss_t = work.tile([P, ct], F32, tag="losst")
                    nc.vector.tensor_reduce(
                        out=loss_t[:],
                        in_=sig_all[:].rearrange("p (j t) -> p j t", t=ct)
                            .transpose([0, 2, 1]),
                        op=mybir.AluOpType.add,
                        axis=mybir.AxisListType.X, negate=True)
                    nc.sync.dma_start(
                        out=loss[:, c * ct:(c + 1) * ct], in_=loss_t[:])
    nc.finalize()
    return nc


def _pack_core_idx(context, target, negatives, ntiles=NTILES, ct=CT):
    """[B_shard,*] int arrays -> [P, nidx] i32.

    Example (c*ct + t)*P + p lives at partition p, chunk c, tile-slot t.
    ctx region per chunk is position-major [CTX, ct]; tgt is [ct];
    neg region per chunk is k-major [K, ct].
    """
    nchunk = ntiles // ct
    ctx_idx = (context.reshape(nchunk, ct, P, CTX)
               .transpose(2, 0, 3, 1).reshape(P, ntiles * CTX))
    tgt_idx = target.reshape(nchunk, ct, P).transpose(2, 0, 1).reshape(P, ntiles)
    neg_idx = (negatives.reshape(nchunk, ct, P, K)
               .transpose(2, 0, 3, 1).reshape(P, ntiles * K))
    return np.ascontiguousarray(
        np.concatenate([ctx_idx, tgt_idx, neg_idx], axis=1).astype(np.int32))


def _run(inputs, trace=False):
    in_embed = np.ascontiguousarray(np.asarray(inputs["in_embed"], dtype=np.float32))
    out_embed = np.ascontiguousarray(np.asarray(inputs["out_embed"], dtype=np.float32))
    context = np.asarray(inputs["context"]).astype(np.int32)
    target = np.asarray(inputs["target"]).astype(np.int32)
    negatives = np.asarray(inputs["negatives"]).astype(np.int32)
    assert context.shape == (B, CTX) and target.shape == (B,) and negatives.shape == (B, K)

    nc = build()
    in_maps = []
    for i in range(NCORES):
        sl = slice(i * B_SHARD, (i + 1) * B_SHARD)
        in_maps.append({
            "in_embed": in_embed,
            "out_embed": out_embed,
            "idx": _pack_core_idx(context[sl], target[sl], negatives[sl]),
        })
    res = run_bass_kernel_spmd(nc, in_maps, core_ids=list(range(NCORES)),
                               trace=trace)
    loss = np.concatenate(
        [res.results[i]["loss"].T.reshape(-1) for i in range(NCORES)])
    return loss.astype(np.float32), res


def kernel(**inputs) -> np.ndarray:
    return _run(inputs, trace=False)[0]
